# revision 1
# baseline (speedup 1.0000x reference)
"""Maxwell rheological model kernel for Trainium2 (8 NeuronCores, SPMD).

Recurrence per batch row (a = E/ETA = 2, E_INFTY = 1, E = 2):
    gamma[0] = 0
    gamma[n+1] = (1 - 2*dt[n]) * gamma[n] + 2*dt[n] * eps[n]
    sigma[n+1] = 3*eps[n+1] - 2*gamma[n+1];  sigma[0] = 0

Mapped onto the DVE TensorTensorScan instruction with g = 2*gamma:
    c[n] = 1 - 2*dt[n]          (ACT engine)
    d[n] = 4*dt[n]*eps[n]       (DVE scalar_tensor_tensor)
    g[n] = c[n]*g[n-1] + d[n]   (DVE tensor_tensor_scan, init 0)
    sigma[:, 1:] = 3*eps[:, 1:] - g[:, :-1]

Batch is sharded across 8 cores (data parallel, no collectives).
"""

import sys

if "/opt/trn_rl_repo" not in sys.path:
    sys.path.insert(0, "/opt/trn_rl_repo")

import numpy as np

import concourse.bacc as bacc
import concourse.mybir as mybir
from concourse.bass_utils import run_bass_kernel_spmd
from concourse.tile import TileContext

B, T = 16384, 2048
N_CORES = 8
B_CORE = B // N_CORES
P = 128
N_STRIPS = B_CORE // P

_prog = None


def _build():
    f32 = mybir.dt.float32
    Alu = mybir.AluOpType
    nc = bacc.Bacc("TRN2", target_bir_lowering=False, debug=False, num_devices=N_CORES)
    strains = nc.dram_tensor("strains", [B_CORE, T], f32, kind="ExternalInput").ap()
    dts = nc.dram_tensor("dts", [B_CORE, T], f32, kind="ExternalInput").ap()
    out = nc.dram_tensor("out", [B_CORE, T], f32, kind="ExternalOutput").ap()
    with TileContext(nc) as tc:
        with tc.tile_pool(name="pool", bufs=3) as pool:
            for i in range(N_STRIPS):
                r0 = i * P
                dt_t = pool.tile([P, T], f32, tag="dt")
                ep_t = pool.tile([P, T], f32, tag="eps")
                nc.sync.dma_start(out=dt_t[:], in_=dts[r0 : r0 + P])
                nc.sync.dma_start(out=ep_t[:], in_=strains[r0 : r0 + P])
                c_t = pool.tile([P, T - 1], f32, tag="c")
                d_t = pool.tile([P, T - 1], f32, tag="d")
                g_t = pool.tile([P, T - 1], f32, tag="g")
                s_t = pool.tile([P, T], f32, tag="sig")
                nc.scalar.activation(
                    out=c_t[:],
                    in_=dt_t[:, : T - 1],
                    func=mybir.ActivationFunctionType.Copy,
                    scale=-2.0,
                    bias=1.0,
                )
                nc.vector.scalar_tensor_tensor(
                    out=d_t[:],
                    in0=dt_t[:, : T - 1],
                    scalar=4.0,
                    in1=ep_t[:, : T - 1],
                    op0=Alu.mult,
                    op1=Alu.mult,
                )
                nc.vector.tensor_tensor_scan(
                    out=g_t[:],
                    data0=c_t[:],
                    data1=d_t[:],
                    initial=0.0,
                    op0=Alu.mult,
                    op1=Alu.add,
                )
                nc.vector.scalar_tensor_tensor(
                    out=s_t[:, 1:],
                    in0=ep_t[:, 1:],
                    scalar=3.0,
                    in1=g_t[:],
                    op0=Alu.mult,
                    op1=Alu.subtract,
                )
                nc.gpsimd.memset(s_t[:, 0:1], 0.0)
                nc.sync.dma_start(out=out[r0 : r0 + P], in_=s_t[:])
    nc.compile()
    return nc


def _get_prog():
    global _prog
    if _prog is None:
        _prog = _build()
    return _prog


def _run(strains, dts, **kwargs):
    nc = _get_prog()
    ss = np.split(np.ascontiguousarray(strains, dtype=np.float32), N_CORES, axis=0)
    ds = np.split(np.ascontiguousarray(dts, dtype=np.float32), N_CORES, axis=0)
    in_maps = [{"strains": s, "dts": d} for s, d in zip(ss, ds)]
    res = run_bass_kernel_spmd(nc, in_maps, core_ids=list(range(N_CORES)), **kwargs)
    full = np.concatenate([r["out"] for r in res.results], axis=0)
    return full, res


def kernel(strains, dts):
    out, _ = _run(strains, dts)
    return out


if __name__ == "__main__":
    rng = np.random.default_rng(0)
    eps = rng.standard_normal((B, T), dtype=np.float32)
    dts = rng.random((B, T), dtype=np.float32)
    out = kernel(eps, dts)
    print("ran ok", out.shape, out.dtype)
